# revision 12
# baseline (speedup 1.0000x reference)
"""GCNConv kernel for 8 Trainium2 NeuronCores — gather-free design.

The edge gather (the baseline bottleneck: SWDGE descriptor emission at
~8ns/row capped the old kernel at ~25 GB/s/core) is eliminated entirely.
The host expands x rows per edge destination (a pure permutation of the
input, like the baseline's shard packing) into a position stream laid out
in run-class order, so the device never does a data-dependent access:

  - Nodes are dealt to cores round-robin in out-degree order, so every
    core sees a near-identical degree distribution (minimal class pad).
    Edges are owned by their src node's core.
  - Per core, each src node's edges form one run padded to an even class
    length R; runs are packed into 512-position bins (one class per bin),
    and bins are processed in same-class pairs: bin A on PSUM partitions
    0:64, bin B on 64:128, so every engine op runs 128 partitions wide.
  - PE computes h^T = theta^T @ xE^T per 512-column block; norm[dst] is
    broadcast across the 64 output channels with one DVE stream_shuffle
    (norm strips seeded at partitions 0/32/64/96, mask=[0]*32).
  - DVE multiplies (PSUM x SBUF, chunk-wide) and segment-reduces each
    pair with one strided pairwise add plus one tensor_reduce(axis=X).
  - out = norm^2 * h_own + norm * agg, with norm/norm^2 shipped from the
    host replicated across the 64 channels (tiny, per-node).

No AllGather / collectives: cores are fully independent; the host splits
edges and reassembles the output rows.
"""

import sys

sys.path.insert(0, "/opt/trn_rl_repo")

import numpy as np
import ml_dtypes

import concourse.bacc as bacc
import concourse.tile as tile
import concourse.mybir as mybir
from concourse import bass_utils

F32 = mybir.dt.float32
BF16 = mybir.dt.bfloat16
bf16 = ml_dtypes.bfloat16

N_NODES = 100000
IN_CH = 256
OUT_CH = 64
N_CORES = 8
NLOC = N_NODES // N_CORES                   # 12500 nodes per core
BIN = 512                                   # positions per bin
CHUNK_PAIRS = 4                             # bin-pairs per psum chunk

_CACHE = {}


def _build(key):
    NPAIR, AGGW, pairR, pairNR = key
    NPOSH = NPAIR * BIN            # positions per half-stream
    aggcol = np.concatenate([[0], np.cumsum(pairNR)])[:-1]
    OWN_BLK = AGGW // BIN
    OWN_CHUNKS = -(-OWN_BLK // CHUNK_PAIRS)
    NPAIR_P = -(-NPAIR // CHUNK_PAIRS) * CHUNK_PAIRS
    TOTPOS = OWN_CHUNKS * CHUNK_PAIRS * 2 * BIN + NPAIR_P * 2 * BIN

    nc = bacc.Bacc("TRN2", target_bir_lowering=False, debug=False,
                   num_devices=N_CORES)
    NCHTOT = TOTPOS // (CHUNK_PAIRS * 2 * BIN)
    xet = nc.dram_tensor("xet", [NCHTOT * 128, 2 * CHUNK_PAIRS * 2 * BIN],
                         BF16, kind="ExternalInput")
    th = nc.dram_tensor("th", [128, 2, OUT_CH], BF16, kind="ExternalInput")
    nrmE = nc.dram_tensor("nrmE", [2, NPOSH], BF16, kind="ExternalInput")
    nC = nc.dram_tensor("nC", [128, AGGW], BF16, kind="ExternalInput")
    outd = nc.dram_tensor("out", [128, AGGW], BF16, kind="ExternalOutput")

    Copy = mybir.ActivationFunctionType.Copy
    ADD = mybir.AluOpType.add
    MULT = mybir.AluOpType.mult

    with tile.TileContext(nc) as tc:
        with tc.tile_pool(name="persist", bufs=1) as pp:
            th_sb = pp.tile([128, 2, OUT_CH], BF16)
            mT = pp.tile([128, AGGW], BF16)
            aggT = pp.tile([128, AGGW], F32)
            nC_sb = pp.tile([128, AGGW], BF16)
            nC2_sb = pp.tile([128, AGGW], BF16)
            nc.sync.dma_start(th_sb[:], th[:])
            nc.vector.memset(aggT[:], 0)

            # ---- unified chunk pipeline: own-node chunks then edge chunks --
            shuf_mask = [0] * 32
            with (
                tc.tile_pool(name="xc", bufs=4) as xcp,
                tc.tile_pool(name="ne", bufs=2) as nep,
                tc.tile_pool(name="nbc", bufs=2) as nbcp,
                tc.tile_pool(name="msg", bufs=2) as msgp,
                tc.tile_pool(name="hps", bufs=2, space="PSUM") as hps,
            ):
                W = CHUNK_PAIRS * BIN
                nch = 0
                for ch in range(OWN_CHUNKS + NPAIR_P // CHUNK_PAIRS):
                    own = ch < OWN_CHUNKS
                    xcs = xcp.tile([128, 4 * W], BF16, tag="xc")
                    nc.sync.dma_start(
                        xcs[:], xet[ch * 128:(ch + 1) * 128, :])
                    if not own:
                        p0 = (ch - OWN_CHUNKS) * CHUNK_PAIRS
                        if p0 >= NPAIR:
                            continue
                        ncp = min(CHUNK_PAIRS, NPAIR - p0)
                        nes = nep.tile([128, W], BF16, tag="ne")
                        if nch < 2:
                            nc.vector.memset(nes[:], 0)
                        nch += 1
                        for row, r0 in ((0, 0), (32, 0), (64, 1), (96, 1)):
                            nc.scalar.dma_start(
                                nes[row:row + 1, 0:ncp * BIN],
                                nrmE[r0:r0 + 1, p0 * BIN:(p0 + ncp) * BIN])
                    else:
                        ncp = CHUNK_PAIRS
                    ph = hps.tile([128, W], F32)
                    for i in range(ncp):
                        co = 2 * BIN * i
                        sl = slice(i * BIN, (i + 1) * BIN)
                        nc.tensor.matmul(ph[0:64, sl], lhsT=th_sb[:, 0, :],
                                         rhs=xcs[:, co:co + BIN],
                                         start=True, stop=False)
                        nc.tensor.matmul(ph[0:64, sl], lhsT=th_sb[:, 1, :],
                                         rhs=xcs[:, 2 * W + co:2 * W + co + BIN],
                                         start=False, stop=True)
                        nc.tensor.matmul(ph[64:128, sl], lhsT=th_sb[:, 0, :],
                                         rhs=xcs[:, co + BIN:co + 2 * BIN],
                                         start=True, stop=False)
                        nc.tensor.matmul(ph[64:128, sl], lhsT=th_sb[:, 1, :],
                                         rhs=xcs[:, 2 * W + co + BIN:2 * W + co + 2 * BIN],
                                         start=False, stop=True)
                    if own:
                        c0 = ch * 2 * W
                        ww = min(AGGW - ch * W, W)
                        nc.scalar.activation(mT[:, ch * W:ch * W + ww],
                                             ph[:, 0:ww], Copy)
                        continue
                    nbc = nbcp.tile([128, W], BF16, tag="nbc")
                    nc.vector.stream_shuffle(nbc[:], nes[:], shuf_mask)
                    msgc = msgp.tile([128, W], BF16, tag="msg")
                    nc.vector.tensor_tensor(msgc[:], ph[:], nbc[:], op=MULT)
                    i = 0
                    while i < ncp:
                        R = int(pairR[p0 + i])
                        nr = int(pairNR[p0 + i])
                        a0 = int(aggcol[p0 + i])
                        if nr == 0:
                            i += 1
                            continue
                        g = 1
                        if nr * R == BIN:
                            while (i + g < ncp
                                   and int(pairR[p0 + i + g]) == R
                                   and int(pairNR[p0 + i + g]) == nr):
                                g += 1
                        seg = msgc[:, i * BIN:i * BIN + (g - 1) * BIN + nr * R]
                        tot = g * nr
                        if R == 2:
                            v = seg.rearrange("p (n t) -> p n t", t=2)
                            nc.vector.tensor_tensor(
                                aggT[:, a0:a0 + tot], v[:, :, 0], v[:, :, 1],
                                op=ADD)
                        else:
                            v = seg.rearrange("p (n r) -> p n r", r=R)
                            nc.vector.tensor_reduce(
                                aggT[:, a0:a0 + tot], v,
                                mybir.AxisListType.X, ADD)
                        i += g

            # ---- Final: out = n2*h_own + n*agg ----
            nc.scalar.dma_start(nC_sb[:], nC[:])
            nc.vector.tensor_tensor(nC2_sb[:], nC_sb[:], nC_sb[:], op=MULT)
            with tc.tile_pool(name="fin", bufs=3) as finp:
                for j in range(AGGW // BIN):
                    sl = slice(j * BIN, (j + 1) * BIN)
                    t1 = finp.tile([128, BIN], F32, tag="t1")
                    t2 = finp.tile([128, BIN], F32, tag="t2")
                    t3 = finp.tile([128, BIN], BF16, tag="t3")
                    nc.vector.tensor_tensor(t1[:], mT[:, sl], nC2_sb[:, sl],
                                            op=MULT)
                    nc.vector.tensor_tensor(t2[:], aggT[:, sl], nC_sb[:, sl],
                                            op=MULT)
                    nc.vector.tensor_tensor(t3[:], t1[:], t2[:], op=ADD)
                    nc.scalar.dma_start(outd[:, sl], t3[:])
    nc.compile()
    return nc


def _prepare(x, theta, edge_index):
    src = np.asarray(edge_index[0], dtype=np.int64)
    dst = np.asarray(edge_index[1], dtype=np.int64)

    degc = np.bincount(src, minlength=N_NODES)       # out-degree
    deg = 1.0 + degc
    norm = (1.0 / np.sqrt(deg)).astype(np.float32)
    normz = np.concatenate([norm, [0.0]]).astype(np.float32)

    # deal nodes to cores round-robin in degree order -> balanced classes
    order_nodes = np.argsort(-degc, kind="stable")
    node_core = np.empty(N_NODES, dtype=np.int64)
    node_lid = np.empty(N_NODES, dtype=np.int64)
    ranks = np.arange(N_NODES)
    node_core[order_nodes] = ranks % N_CORES
    node_lid[order_nodes] = ranks // N_CORES
    core_nodes = np.empty((N_CORES, NLOC), dtype=np.int64)
    core_nodes[node_core[order_nodes], node_lid[order_nodes]] = order_nodes

    x_bf = np.asarray(x, dtype=np.float32).astype(bf16)
    xz = np.vstack([x_bf, np.zeros((1, IN_CH), dtype=bf16)])
    # [128, 2, N+1]: [p, h, n] = x[n, h*128+p]
    xTp = np.ascontiguousarray(xz.T.reshape(2, 128, N_NODES + 1)
                               .transpose(1, 0, 2))
    th_pack = np.ascontiguousarray(
        np.asarray(theta, dtype=np.float32).astype(bf16)
        .reshape(2, 128, OUT_CH).transpose(1, 0, 2))

    # per-core run structure
    cores = []
    for k in range(N_CORES):
        m = node_core[src] == k
        sK = node_lid[src[m]]
        dK = dst[m]
        cnt = np.bincount(sK, minlength=NLOC)
        R = cnt + (cnt & 1)
        assert R.max() <= BIN, f"run too long: {R.max()}"
        cores.append((sK, dK, cnt, R))

    # unified class structure: for each even R, bins = max over cores,
    # padded to an even bin count (same-class pairs)
    all_R = sorted(set(int(r) for (_, _, cnt, R) in cores
                       for r in np.unique(R[cnt > 0])))
    classes = []           # (R, nbins, runs_per_bin)
    for Rv in all_R:
        rpb = BIN // Rv
        nb = 0
        for (_, _, cnt, R) in cores:
            nr = int(np.count_nonzero((R == Rv) & (cnt > 0)))
            nb = max(nb, -(-nr // rpb))
        nb += nb & 1
        classes.append((Rv, nb, rpb))

    NBINS = sum(nb for (_, nb, _) in classes)
    NPAIR = NBINS // 2
    pairR = []
    pairNR = []
    for (Rv, nb, rpb) in classes:
        pairR += [Rv] * (nb // 2)
        pairNR += [rpb] * (nb // 2)
    npad_pairs = (-(-NPAIR // CHUNK_PAIRS) * CHUNK_PAIRS) - NPAIR
    pairR += [2] * npad_pairs
    pairNR += [0] * npad_pairs
    NPAIR += npad_pairs
    NBINS = 2 * NPAIR
    run_cols = sum(pairNR)
    max_inact = max(int(np.count_nonzero(cnt == 0))
                    for (_, _, cnt, _) in cores)
    AGGW = -(-(run_cols + -(-max_inact // 2)) // BIN) * BIN
    key = (NPAIR, AGGW, tuple(pairR), tuple(pairNR))

    in_maps = []
    node_maps = []
    for k in range(N_CORES):
        sK, dK, cnt, R = cores[k]
        # edge order: by (R class, lid); within a node keep input order
        eord = np.lexsort((sK, R[sK]))
        sK_s = sK[eord]
        dK_s = dK[eord]

        dstpos = np.full(NBINS * BIN, N_NODES, dtype=np.int64)
        nodeA = np.full(AGGW, -1, dtype=np.int64)   # local lids, -1 = dummy
        nodeB = np.full(AGGW, -1, dtype=np.int64)
        binbase = 0
        e0 = 0
        a0 = 0
        for (Rv, nb, rpb) in classes:
            sel = np.flatnonzero((R == Rv) & (cnt > 0))    # lids, asc
            nrn = len(sel)
            necls = int(cnt[sel].sum())
            # run r -> bin r//rpb, slot (r%rpb)*Rv
            r_idx = np.arange(nrn)
            start = (binbase + r_idx // rpb) * BIN + (r_idx % rpb) * Rv
            rep = np.repeat(np.arange(nrn), cnt[sel])
            off = np.concatenate([[0], np.cumsum(cnt[sel])])[:-1]
            rank = np.arange(necls) - np.repeat(off, cnt[sel])
            dstpos[start[rep] + rank] = dK_s[e0:e0 + necls]
            e0 += necls
            # node lists: bin b holds runs b*rpb..(b+1)*rpb (pad -1)
            nodes_pad = np.full(nb * rpb, -1, dtype=np.int64)
            nodes_pad[:nrn] = sel
            nodes_pad = nodes_pad.reshape(nb, rpb)
            npair_c = nb // 2
            nodeA[a0:a0 + npair_c * rpb] = nodes_pad[0::2].ravel()
            nodeB[a0:a0 + npair_c * rpb] = nodes_pad[1::2].ravel()
            a0 += npair_c * rpb
            binbase += nb
        assert e0 == len(sK_s)
        # inactive nodes appended after run columns
        inact = np.flatnonzero(cnt == 0)
        h1 = -(-len(inact) // 2)
        nodeA[a0:a0 + h1] = inact[:h1]
        nodeB[a0:a0 + len(inact) - h1] = inact[h1:]

        gidA = np.where(nodeA >= 0, core_nodes[k][nodeA], N_NODES)
        gidB = np.where(nodeB >= 0, core_nodes[k][nodeB], N_NODES)

        # own-node blocks prepended as ordinary chunks (A|B alternating)
        OWN_BLK = AGGW // BIN
        OWN_CHUNKS = -(-OWN_BLK // CHUNK_PAIRS)
        gid_own = np.full(OWN_CHUNKS * CHUNK_PAIRS * 2 * BIN, N_NODES,
                          dtype=np.int64)
        inter = np.stack([gidA.reshape(OWN_BLK, BIN),
                          gidB.reshape(OWN_BLK, BIN)], axis=1).ravel()
        gid_own[:inter.shape[0]] = inter
        allpos = np.concatenate([gid_own, dstpos])
        xe_g = xTp[:, :, allpos]                      # [128, 2, TOTPOS]
        nchk = allpos.shape[0] // (2 * CHUNK_PAIRS * BIN)
        xet = np.ascontiguousarray(
            xe_g.reshape(128, 2, nchk, 2 * CHUNK_PAIRS * BIN)
            .transpose(2, 0, 1, 3).reshape(nchk * 128, -1))
        npos = normz[dstpos].reshape(NBINS, BIN)
        nrmE_v = np.ascontiguousarray(
            np.stack([npos[0::2].ravel(), npos[1::2].ravel()])).astype(bf16)

        nA = normz[gidA].astype(np.float32)
        nB = normz[gidB].astype(np.float32)
        nC_v = np.empty((128, AGGW), dtype=bf16)
        nC_v[0:64] = nA[None, :].astype(bf16)
        nC_v[64:128] = nB[None, :].astype(bf16)

        in_maps.append({
            "xet": xet, "th": th_pack, "nrmE": nrmE_v, "nC": nC_v,
        })
        node_maps.append((gidA, gidB))
    return in_maps, (key, node_maps)


def _assemble(results, node_maps):
    out = np.empty((N_NODES, OUT_CH), dtype=np.float32)
    for k in range(N_CORES):
        gidA, gidB = node_maps[k]
        op = results[k]["out"].astype(np.float32)   # [128, AGGW]
        va = gidA < N_NODES
        vb = gidB < N_NODES
        out[gidA[va]] = op[0:64, va].T
        out[gidB[vb]] = op[64:128, vb].T
    return out


def kernel(x, theta, edge_index):
    in_maps, (key, node_maps) = _prepare(x, theta, edge_index)
    if key not in _CACHE:
        _CACHE[key] = _build(key)
    nc = _CACHE[key]
    res = bass_utils.run_bass_kernel_spmd(
        nc, in_maps, core_ids=list(range(N_CORES)))
    return _assemble(res.results, node_maps)


# revision 14
# speedup vs baseline: 1.0316x; 1.0316x over previous
"""GCNConv kernel for 8 Trainium2 NeuronCores — gather-free design.

The edge gather (the baseline bottleneck: SWDGE descriptor emission at
~8ns/row capped the old kernel at ~25 GB/s/core) is eliminated entirely.
The host expands x rows per edge destination (a pure permutation of the
input, like the baseline's shard packing) into a position stream laid out
in run-class order, so the device never does a data-dependent access:

  - Nodes are dealt to cores round-robin in out-degree order, so every
    core sees a near-identical degree distribution (minimal class pad).
    Edges are owned by their src node's core.
  - Per core, each src node's edges form one run padded to an even class
    length R; runs are packed into 512-position bins (one class per bin),
    and bins are processed in same-class pairs: bin A on PSUM partitions
    0:64, bin B on 64:128, so every engine op runs 128 partitions wide.
  - PE computes h^T = theta^T @ xE^T per 512-column block; norm[dst] is
    broadcast across the 64 output channels with one DVE stream_shuffle
    (norm strips seeded at partitions 0/32/64/96, mask=[0]*32).
  - DVE multiplies (PSUM x SBUF, chunk-wide) and segment-reduces each
    pair with one strided pairwise add plus one tensor_reduce(axis=X).
  - out = norm^2 * h_own + norm * agg, with norm/norm^2 shipped from the
    host replicated across the 64 channels (tiny, per-node).

No AllGather / collectives: cores are fully independent; the host splits
edges and reassembles the output rows.
"""

import sys

sys.path.insert(0, "/opt/trn_rl_repo")

import numpy as np
import ml_dtypes

import concourse.bacc as bacc
import concourse.tile as tile
import concourse.mybir as mybir
from concourse import bass_utils

F32 = mybir.dt.float32
BF16 = mybir.dt.bfloat16
bf16 = ml_dtypes.bfloat16

N_NODES = 100000
IN_CH = 256
OUT_CH = 64
N_CORES = 8
NLOC = N_NODES // N_CORES                   # 12500 nodes per core
BIN = 512                                   # positions per bin
CHUNK_PAIRS = 4                             # bin-pairs per psum chunk

_CACHE = {}


def _build(key):
    NPAIR, AGGW, pairR, pairNR = key
    NPOSH = NPAIR * BIN            # positions per half-stream
    aggcol = np.concatenate([[0], np.cumsum(pairNR)])[:-1]
    OWN_BLK = AGGW // BIN
    OWN_CHUNKS = -(-OWN_BLK // CHUNK_PAIRS)
    NPAIR_P = -(-NPAIR // CHUNK_PAIRS) * CHUNK_PAIRS
    TOTPOS = OWN_CHUNKS * CHUNK_PAIRS * 2 * BIN + NPAIR_P * 2 * BIN

    nc = bacc.Bacc("TRN2", target_bir_lowering=False, debug=False,
                   num_devices=N_CORES)
    NCHTOT = TOTPOS // (CHUNK_PAIRS * 2 * BIN)
    xet = nc.dram_tensor("xet", [NCHTOT * 128, 2 * CHUNK_PAIRS * 2 * BIN],
                         BF16, kind="ExternalInput")
    th = nc.dram_tensor("th", [128, 2, OUT_CH], BF16, kind="ExternalInput")
    nrmE = nc.dram_tensor("nrmE", [2, NPOSH], BF16, kind="ExternalInput")
    nC = nc.dram_tensor("nC", [128, AGGW], BF16, kind="ExternalInput")
    outd = nc.dram_tensor("out", [128, AGGW], BF16, kind="ExternalOutput")

    Copy = mybir.ActivationFunctionType.Copy
    ADD = mybir.AluOpType.add
    MULT = mybir.AluOpType.mult

    with tile.TileContext(nc) as tc:
        with tc.tile_pool(name="persist", bufs=1) as pp:
            th_sb = pp.tile([128, 2, OUT_CH], BF16)
            mT = pp.tile([128, AGGW], BF16)
            aggT = pp.tile([128, AGGW], F32)
            nC_sb = pp.tile([128, AGGW], BF16)
            nC2_sb = pp.tile([128, AGGW], BF16)
            nc.sync.dma_start(th_sb[:], th[:])
            nc.vector.memset(aggT[:], 0)

            # ---- unified chunk pipeline: own-node chunks then edge chunks --
            shuf_mask = [0] * 32
            with (
                tc.tile_pool(name="xc", bufs=5) as xcp,
                tc.tile_pool(name="ne", bufs=3) as nep,
                tc.tile_pool(name="nbc", bufs=3) as nbcp,
                tc.tile_pool(name="msg", bufs=3) as msgp,
                tc.tile_pool(name="hps", bufs=2, space="PSUM") as hps,
            ):
                W = CHUNK_PAIRS * BIN
                nch = 0
                for ch in range(OWN_CHUNKS + NPAIR_P // CHUNK_PAIRS):
                    own = ch < OWN_CHUNKS
                    xcs = xcp.tile([128, 4 * W], BF16, tag="xc")
                    nc.sync.dma_start(
                        xcs[:], xet[ch * 128:(ch + 1) * 128, :])
                    if not own:
                        p0 = (ch - OWN_CHUNKS) * CHUNK_PAIRS
                        if p0 >= NPAIR:
                            continue
                        ncp = min(CHUNK_PAIRS, NPAIR - p0)
                        nes = nep.tile([128, W], BF16, tag="ne")
                        if nch < 3:
                            nc.vector.memset(nes[:], 0)
                        nch += 1
                        for row, r0 in ((0, 0), (32, 0), (64, 1), (96, 1)):
                            nc.scalar.dma_start(
                                nes[row:row + 1, 0:ncp * BIN],
                                nrmE[r0:r0 + 1, p0 * BIN:(p0 + ncp) * BIN])
                    else:
                        ncp = CHUNK_PAIRS
                    ph = hps.tile([128, W], F32)
                    for i in range(ncp):
                        co = 2 * BIN * i
                        sl = slice(i * BIN, (i + 1) * BIN)
                        nc.tensor.matmul(ph[0:64, sl], lhsT=th_sb[:, 0, :],
                                         rhs=xcs[:, co:co + BIN],
                                         start=True, stop=False)
                        nc.tensor.matmul(ph[0:64, sl], lhsT=th_sb[:, 1, :],
                                         rhs=xcs[:, 2 * W + co:2 * W + co + BIN],
                                         start=False, stop=True)
                        nc.tensor.matmul(ph[64:128, sl], lhsT=th_sb[:, 0, :],
                                         rhs=xcs[:, co + BIN:co + 2 * BIN],
                                         start=True, stop=False)
                        nc.tensor.matmul(ph[64:128, sl], lhsT=th_sb[:, 1, :],
                                         rhs=xcs[:, 2 * W + co + BIN:2 * W + co + 2 * BIN],
                                         start=False, stop=True)
                    if own:
                        c0 = ch * 2 * W
                        ww = min(AGGW - ch * W, W)
                        nc.scalar.activation(mT[:, ch * W:ch * W + ww],
                                             ph[:, 0:ww], Copy)
                        continue
                    nbc = nbcp.tile([128, W], BF16, tag="nbc")
                    nc.vector.stream_shuffle(nbc[:], nes[:], shuf_mask)
                    msgc = msgp.tile([128, W], BF16, tag="msg")
                    nc.vector.tensor_tensor(msgc[:], ph[:], nbc[:], op=MULT)
                    for i in range(ncp):
                        R = int(pairR[p0 + i])
                        nr = int(pairNR[p0 + i])
                        a0 = int(aggcol[p0 + i])
                        if nr == 0:
                            continue
                        seg = msgc[:, i * BIN:i * BIN + nr * R]
                        if R == 2:
                            v = seg.rearrange("p (n t) -> p n t", t=2)
                            nc.vector.tensor_tensor(
                                aggT[:, a0:a0 + nr], v[:, :, 0], v[:, :, 1],
                                op=ADD)
                        else:
                            v = seg.rearrange("p (n r) -> p n r", r=R)
                            nc.vector.tensor_reduce(
                                aggT[:, a0:a0 + nr], v,
                                mybir.AxisListType.X, ADD)

            # ---- Final: out = n2*h_own + n*agg ----
            nc.scalar.dma_start(nC_sb[:], nC[:])
            nc.vector.tensor_tensor(nC2_sb[:], nC_sb[:], nC_sb[:], op=MULT)
            with tc.tile_pool(name="fin", bufs=3) as finp:
                for j in range(AGGW // BIN):
                    sl = slice(j * BIN, (j + 1) * BIN)
                    t1 = finp.tile([128, BIN], F32, tag="t1")
                    t2 = finp.tile([128, BIN], F32, tag="t2")
                    t3 = finp.tile([128, BIN], BF16, tag="t3")
                    nc.vector.tensor_tensor(t1[:], mT[:, sl], nC2_sb[:, sl],
                                            op=MULT)
                    nc.vector.tensor_tensor(t2[:], aggT[:, sl], nC_sb[:, sl],
                                            op=MULT)
                    nc.vector.tensor_tensor(t3[:], t1[:], t2[:], op=ADD)
                    nc.scalar.dma_start(outd[:, sl], t3[:])
    nc.compile()
    return nc


def _prepare(x, theta, edge_index):
    src = np.asarray(edge_index[0], dtype=np.int64)
    dst = np.asarray(edge_index[1], dtype=np.int64)

    degc = np.bincount(src, minlength=N_NODES)       # out-degree
    deg = 1.0 + degc
    norm = (1.0 / np.sqrt(deg)).astype(np.float32)
    normz = np.concatenate([norm, [0.0]]).astype(np.float32)

    # deal nodes to cores round-robin in degree order -> balanced classes
    order_nodes = np.argsort(-degc, kind="stable")
    node_core = np.empty(N_NODES, dtype=np.int64)
    node_lid = np.empty(N_NODES, dtype=np.int64)
    ranks = np.arange(N_NODES)
    node_core[order_nodes] = ranks % N_CORES
    node_lid[order_nodes] = ranks // N_CORES
    core_nodes = np.empty((N_CORES, NLOC), dtype=np.int64)
    core_nodes[node_core[order_nodes], node_lid[order_nodes]] = order_nodes

    x_bf = np.asarray(x, dtype=np.float32).astype(bf16)
    xz = np.vstack([x_bf, np.zeros((1, IN_CH), dtype=bf16)])
    # [128, 2, N+1]: [p, h, n] = x[n, h*128+p]
    xTp = np.ascontiguousarray(xz.T.reshape(2, 128, N_NODES + 1)
                               .transpose(1, 0, 2))
    th_pack = np.ascontiguousarray(
        np.asarray(theta, dtype=np.float32).astype(bf16)
        .reshape(2, 128, OUT_CH).transpose(1, 0, 2))

    # per-core run structure
    cores = []
    for k in range(N_CORES):
        m = node_core[src] == k
        sK = node_lid[src[m]]
        dK = dst[m]
        cnt = np.bincount(sK, minlength=NLOC)
        R = cnt + (cnt & 1)
        assert R.max() <= BIN, f"run too long: {R.max()}"
        cores.append((sK, dK, cnt, R))

    # unified class structure: for each even R, bins = max over cores,
    # padded to an even bin count (same-class pairs)
    all_R = sorted(set(int(r) for (_, _, cnt, R) in cores
                       for r in np.unique(R[cnt > 0])))
    classes = []           # (R, nbins, runs_per_bin)
    for Rv in all_R:
        rpb = BIN // Rv
        nb = 0
        for (_, _, cnt, R) in cores:
            nr = int(np.count_nonzero((R == Rv) & (cnt > 0)))
            nb = max(nb, -(-nr // rpb))
        nb += nb & 1
        classes.append((Rv, nb, rpb))

    NBINS = sum(nb for (_, nb, _) in classes)
    NPAIR = NBINS // 2
    pairR = []
    pairNR = []
    for (Rv, nb, rpb) in classes:
        pairR += [Rv] * (nb // 2)
        pairNR += [rpb] * (nb // 2)
    npad_pairs = (-(-NPAIR // CHUNK_PAIRS) * CHUNK_PAIRS) - NPAIR
    pairR += [2] * npad_pairs
    pairNR += [0] * npad_pairs
    NPAIR += npad_pairs
    NBINS = 2 * NPAIR
    run_cols = sum(pairNR)
    max_inact = max(int(np.count_nonzero(cnt == 0))
                    for (_, _, cnt, _) in cores)
    AGGW = -(-(run_cols + -(-max_inact // 2)) // BIN) * BIN
    key = (NPAIR, AGGW, tuple(pairR), tuple(pairNR))

    in_maps = []
    node_maps = []
    for k in range(N_CORES):
        sK, dK, cnt, R = cores[k]
        # edge order: by (R class, lid); within a node keep input order
        eord = np.lexsort((sK, R[sK]))
        sK_s = sK[eord]
        dK_s = dK[eord]

        dstpos = np.full(NBINS * BIN, N_NODES, dtype=np.int64)
        nodeA = np.full(AGGW, -1, dtype=np.int64)   # local lids, -1 = dummy
        nodeB = np.full(AGGW, -1, dtype=np.int64)
        binbase = 0
        e0 = 0
        a0 = 0
        for (Rv, nb, rpb) in classes:
            sel = np.flatnonzero((R == Rv) & (cnt > 0))    # lids, asc
            nrn = len(sel)
            necls = int(cnt[sel].sum())
            # run r -> bin r//rpb, slot (r%rpb)*Rv
            r_idx = np.arange(nrn)
            start = (binbase + r_idx // rpb) * BIN + (r_idx % rpb) * Rv
            rep = np.repeat(np.arange(nrn), cnt[sel])
            off = np.concatenate([[0], np.cumsum(cnt[sel])])[:-1]
            rank = np.arange(necls) - np.repeat(off, cnt[sel])
            dstpos[start[rep] + rank] = dK_s[e0:e0 + necls]
            e0 += necls
            # node lists: bin b holds runs b*rpb..(b+1)*rpb (pad -1)
            nodes_pad = np.full(nb * rpb, -1, dtype=np.int64)
            nodes_pad[:nrn] = sel
            nodes_pad = nodes_pad.reshape(nb, rpb)
            npair_c = nb // 2
            nodeA[a0:a0 + npair_c * rpb] = nodes_pad[0::2].ravel()
            nodeB[a0:a0 + npair_c * rpb] = nodes_pad[1::2].ravel()
            a0 += npair_c * rpb
            binbase += nb
        assert e0 == len(sK_s)
        # inactive nodes appended after run columns
        inact = np.flatnonzero(cnt == 0)
        h1 = -(-len(inact) // 2)
        nodeA[a0:a0 + h1] = inact[:h1]
        nodeB[a0:a0 + len(inact) - h1] = inact[h1:]

        gidA = np.where(nodeA >= 0, core_nodes[k][nodeA], N_NODES)
        gidB = np.where(nodeB >= 0, core_nodes[k][nodeB], N_NODES)

        # own-node blocks prepended as ordinary chunks (A|B alternating)
        OWN_BLK = AGGW // BIN
        OWN_CHUNKS = -(-OWN_BLK // CHUNK_PAIRS)
        gid_own = np.full(OWN_CHUNKS * CHUNK_PAIRS * 2 * BIN, N_NODES,
                          dtype=np.int64)
        inter = np.stack([gidA.reshape(OWN_BLK, BIN),
                          gidB.reshape(OWN_BLK, BIN)], axis=1).ravel()
        gid_own[:inter.shape[0]] = inter
        allpos = np.concatenate([gid_own, dstpos])
        xe_g = xTp[:, :, allpos]                      # [128, 2, TOTPOS]
        nchk = allpos.shape[0] // (2 * CHUNK_PAIRS * BIN)
        xet = np.ascontiguousarray(
            xe_g.reshape(128, 2, nchk, 2 * CHUNK_PAIRS * BIN)
            .transpose(2, 0, 1, 3).reshape(nchk * 128, -1))
        npos = normz[dstpos].reshape(NBINS, BIN)
        nrmE_v = np.ascontiguousarray(
            np.stack([npos[0::2].ravel(), npos[1::2].ravel()])).astype(bf16)

        nA = normz[gidA].astype(np.float32)
        nB = normz[gidB].astype(np.float32)
        nC_v = np.empty((128, AGGW), dtype=bf16)
        nC_v[0:64] = nA[None, :].astype(bf16)
        nC_v[64:128] = nB[None, :].astype(bf16)

        in_maps.append({
            "xet": xet, "th": th_pack, "nrmE": nrmE_v, "nC": nC_v,
        })
        node_maps.append((gidA, gidB))
    return in_maps, (key, node_maps)


def _assemble(results, node_maps):
    out = np.empty((N_NODES, OUT_CH), dtype=np.float32)
    for k in range(N_CORES):
        gidA, gidB = node_maps[k]
        op = results[k]["out"].astype(np.float32)   # [128, AGGW]
        va = gidA < N_NODES
        vb = gidB < N_NODES
        out[gidA[va]] = op[0:64, va].T
        out[gidB[vb]] = op[64:128, vb].T
    return out


def kernel(x, theta, edge_index):
    in_maps, (key, node_maps) = _prepare(x, theta, edge_index)
    if key not in _CACHE:
        _CACHE[key] = _build(key)
    nc = _CACHE[key]
    res = bass_utils.run_bass_kernel_spmd(
        nc, in_maps, core_ids=list(range(N_CORES)))
    return _assemble(res.results, node_maps)
